# revision 4
# baseline (speedup 1.0000x reference)
"""GridSmoother kernel for 8 trn2 NeuronCores.

Sharding: data-parallel over B (16 samples -> 2 per core). The 12-layer
transformer (the dominant compute) runs on the 8 NeuronCores as a Bass/Tile
program with fp32r matmuls; weights are uploaded sharded 8-ways and
AllGathered on-device to cut host->device transfer 8x. FPS runs on the host
overlapped with the device call; chamfer/homogeneity are vectorized numpy.

Environment notes (this container):
- walrus only accepts 1 sync-wait per instruction (2 on EventSemaphore); a
  BIR post-pass splits multi-wait instructions into EventSemaphore prefixes.
- Compiled NEFFs are content-cached under /tmp/bass_neff_cache.
"""

import json
import os
import sys
import threading

import numpy as np
from contextlib import ExitStack

sys.path.insert(0, "/opt/trn_rl_repo")

B, P, N, D, L, H = 16, 8192, 1024, 384, 12, 6
HD = D // H
K_NEI = 5
DC = D // 128
NC8 = N // 128
HALF = 512

# ---------------------------------------------------------------------------
# BIR fix: split multi-wait instructions (this container's walrus build only
# supports one sync-wait command per instruction), plus a NEFF content cache.
# ---------------------------------------------------------------------------

_NEFF_CACHE = "/tmp/bass_neff_cache"


def _split_waits_json(bir_json):
    d = json.loads(bir_json)
    for fn in d.get("functions", []):
        for bb in fn.get("blocks", []):
            out = []
            for ins in bb.get("instructions", []):
                si = ins.get("sync_info")
                waits = (si or {}).get("on_wait") or []
                cap = 2 if ins.get("opcode") == "EventSemaphore" else 1
                if len(waits) > cap:
                    extra, keep = waits[:-cap], waits[-cap:]
                    for gi in range(0, len(extra), 2):
                        out.append({
                            "debug": ins.get("debug"),
                            "engine": ins["engine"],
                            "ins": [], "outs": [],
                            "name": f"{ins['name']}-sw{gi}",
                            "opcode": "EventSemaphore",
                            "sync_info": {"on_update": [], "on_wait": extra[gi:gi+2]},
                        })
                    si["on_wait"] = keep
                out.append(ins)
            bb["instructions"] = out
    return json.dumps(d).encode()


def _install_birfix():
    import concourse.bass_utils as bu
    import concourse.bass2jax as b2j
    import hashlib
    import shutil
    if getattr(bu, "_birfix_installed", False):
        return
    orig = bu.compile_bir_kernel

    def patched(bir_json, tmpdir, neff_name="file.neff"):
        fixed = _split_waits_json(bir_json)
        os.makedirs(_NEFF_CACHE, exist_ok=True)
        key = hashlib.sha256(fixed).hexdigest()[:32]
        cpath = os.path.join(_NEFF_CACHE, key + ".neff")
        dst = os.path.join(tmpdir, neff_name)
        if os.path.exists(cpath):
            shutil.copyfile(cpath, dst)
            return dst
        out = orig(fixed, tmpdir, neff_name=neff_name)
        try:
            shutil.copyfile(out, cpath)
        except OSError:
            pass
        return out

    bu.compile_bir_kernel = patched
    b2j.compile_bir_kernel = patched
    bu._birfix_installed = True


# ---------------------------------------------------------------------------
# Device program: AllGather weights + embed + 12 transformer layers + proj
# ---------------------------------------------------------------------------

SZ_QKV = D * 3 * D
SZ_WO = D * D
SZ_W1 = D * 4 * D
SZ_W2 = 4 * D * D
SZ_BIAS = 128 * 27
SZ_VB = D
OFF_QKV = 0
OFF_WO = OFF_QKV + L * SZ_QKV
OFF_W1 = OFF_WO + L * SZ_WO
OFF_W2 = OFF_W1 + L * SZ_W1
OFF_BIAS = OFF_W2 + L * SZ_W2
OFF_VB = OFF_BIAS + L * SZ_BIAS
OFF_EMB = OFF_VB + L * SZ_VB
OFF_PROJ = OFF_EMB + 3 * D
TOT_RAW = OFF_PROJ + D * 3
TOT = ((TOT_RAW + 1023) // 1024) * 1024
SHARD = TOT // 8

_NC_CACHE = {}


def _fold_weights(inputs):
    """Fold LN scales into matmul weights, 1/sqrt(HD) into q."""
    f64 = lambda x: np.asarray(x, np.float64)
    ln1w, ln1b = f64(inputs["ln1_w"]), f64(inputs["ln1_b"])
    ln2w, ln2b = f64(inputs["ln2_w"]), f64(inputs["ln2_b"])
    qkvw, qkvb = f64(inputs["qkv_w"]), f64(inputs["qkv_b"])
    m1w, m1b = f64(inputs["mlp_w1"]), f64(inputs["mlp_b1"])
    Wq = ln1w[:, :, None] * qkvw
    bq = np.einsum("ld,lde->le", ln1b, qkvw) + qkvb
    Wq[:, :, :D] *= 0.125
    bq[:, :D] *= 0.125
    W1 = ln2w[:, :, None] * m1w
    b1 = np.einsum("ld,lde->le", ln2b, m1w) + m1b
    Wo = f64(inputs["attn_w"]); ab = f64(inputs["attn_b"])
    W2 = f64(inputs["mlp_w2"]); b2 = f64(inputs["mlp_b2"])
    bias_all = np.zeros((L, 128, 27), np.float64)
    for l in range(L):
        cols = np.concatenate([bq[l], ab[l], b1[l], b2[l]])
        bias_all[l] = cols.reshape(27, 128).T
    blob = np.zeros(TOT, np.float32)

    def put(off, a):
        a = np.ascontiguousarray(np.asarray(a, np.float32)).ravel()
        blob[off:off + a.size] = a

    put(OFF_QKV, Wq); put(OFF_WO, Wo); put(OFF_W1, W1); put(OFF_W2, W2)
    put(OFF_BIAS, bias_all); put(OFF_VB, bq[:, 2 * D:3 * D])
    put(OFF_EMB, inputs["embed_w"]); put(OFF_PROJ, inputs["proj_w"])
    return blob.reshape(8, SHARD)


def _build_transformer_nc():
    import concourse.bass as bass
    import concourse.tile as tile
    from concourse import mybir
    from concourse import masks

    F32 = mybir.dt.float32
    F32R = mybir.dt.float32r
    AF = mybir.ActivationFunctionType
    OP = mybir.AluOpType

    def r(t):
        return t.bitcast(F32R)

    nc = bass.Bass("TRN2", target_bir_lowering=False, debug=False, num_devices=8)
    grid_d = nc.dram_tensor("grid2", [2, N, 3], F32, kind="ExternalInput").ap()
    wsh_d = nc.dram_tensor("wshard", [SHARD], F32, kind="ExternalInput").ap()
    pred_d = nc.dram_tensor("predT", [2, 3, N], F32, kind="ExternalOutput").ap()
    wfull = nc.dram_tensor("wfull", [TOT], F32, kind="Internal",
                           addr_space="Shared").ap()
    wtmp = nc.dram_tensor("wtmp", [SHARD], F32, kind="Internal").ap()

    with tile.TileContext(nc) as tc, ExitStack() as ctx, \
            nc.allow_low_precision(reason="f32r rounding for fast matmul is intended"):
        nc.sync.dma_start(wtmp[:], wsh_d[:])
        nc.gpsimd.collective_compute(
            "AllGather", mybir.AluOpType.bypass,
            replica_groups=[[0, 1, 2, 3, 4, 5, 6, 7]],
            ins=[wtmp[:].opt()], outs=[wfull[:].opt()])
        cst = ctx.enter_context(tc.tile_pool(name="cst", bufs=1))
        sb = ctx.enter_context(tc.tile_pool(name="sb", bufs=2))
        ps = ctx.enter_context(tc.tile_pool(name="ps", bufs=2, space="PSUM"))
        big = ctx.enter_context(tc.tile_pool(name="big", bufs=1))
        wp = ctx.enter_context(tc.tile_pool(name="wpool", bufs=1))
        xp = ctx.enter_context(tc.tile_pool(name="xpool", bufs=1))

        ident = cst.tile([128, 128], F32)
        masks.make_identity(nc, ident[:])
        ones1 = cst.tile([1, 128], F32)
        nc.vector.memset(ones1[:], 1.0)
        ones128 = cst.tile([128, 1], F32)
        nc.vector.memset(ones128[:], 1.0)
        eps1 = cst.tile([1, 1], F32)
        nc.vector.memset(eps1[:], 1e-5)

        def psA(shape=(128, HALF)):
            return ps.tile(list(shape), F32, name="psA", tag="psA")

        embw = cst.tile([3, D], F32)
        nc.sync.dma_start(r(embw[:]), r(wfull[OFF_EMB:OFF_EMB + 3 * D]
                                        .rearrange("(a b) -> a b", a=3)))
        projw = cst.tile([128, DC, 3], F32)
        nc.sync.dma_start(r(projw[:]), r(wfull[OFF_PROJ:OFF_PROJ + D * 3]
                                         .rearrange("(c p k) -> p c k", c=DC, p=128)))
        biases = cst.tile([128, L, 27], F32)
        nc.sync.dma_start(biases[:], wfull[OFF_BIAS:OFF_BIAS + L * SZ_BIAS]
                          .rearrange("(l p c) -> p l c", l=L, p=128))

        def layernorm(xT, hT):
            sq = big.tile([128, DC, N], F32, name="ln_sq", tag="ln_sq")
            nc.vector.tensor_tensor(r(sq[:]), xT[:], xT[:], OP.mult)
            mean = sb.tile([1, N], F32, name="ln_mean", tag="ln_mean", bufs=1)
            veps = sb.tile([1, N], F32, name="ln_veps", tag="ln_veps", bufs=1)
            for h in range(2):
                hs = slice(h * HALF, (h + 1) * HALF)
                s1 = psA((1, HALF))
                s2 = psA((1, HALF))
                for c in range(DC):
                    nc.tensor.matmul(s1[:], r(ones128[:]), r(xT[:, c, hs]),
                                     start=(c == 0), stop=(c == DC - 1))
                    nc.tensor.matmul(s2[:], r(ones128[:]), r(sq[:, c, hs]),
                                     start=(c == 0), stop=(c == DC - 1))
                nc.vector.tensor_scalar(r(mean[:, hs]), s1[:], 1.0 / D, None, op0=OP.mult)
                msq = sb.tile([1, HALF], F32, name="ln_msq", tag="ln_msq", bufs=1)
                nc.vector.tensor_tensor(msq[:], mean[:, hs], mean[:, hs], OP.mult)
                nc.vector.scalar_tensor_tensor(veps[:, hs], s2[:], 1.0 / D, msq[:],
                                               op0=OP.mult, op1=OP.subtract)
            stdev = sb.tile([1, N], F32, name="ln_stdev", tag="ln_stdev", bufs=1)
            nc.scalar.activation(stdev[:], veps[:], AF.Sqrt, bias=eps1[0:1, 0:1], scale=1.0)
            inv = sb.tile([1, N], F32, name="ln_inv", tag="ln_inv", bufs=1)
            nc.vector.reciprocal(r(inv[:]), stdev[:])
            for h in range(2):
                hs = slice(h * HALF, (h + 1) * HALF)
                pmean = psA()
                nc.tensor.matmul(pmean[:], r(ones1[:]), r(mean[:, hs]), start=True, stop=True)
                for c in range(DC):
                    nc.vector.tensor_tensor(r(hT[:, c, hs]), xT[:, c, hs], pmean[:], OP.subtract)
                pinv = psA()
                nc.tensor.matmul(pinv[:], r(ones1[:]), r(inv[:, hs]), start=True, stop=True)
                for c in range(DC):
                    nc.vector.tensor_tensor(r(hT[:, c, hs]), hT[:, c, hs], pinv[:], OP.mult)

        # ---- embed both samples ----
        xT_list = []
        for s in range(2):
            xT = xp.tile([128, DC, N], F32, name=f"xT{s}", tag=f"xT{s}")
            xT_list.append(xT)
            gall = sb.tile([128, 8, 3], F32, name="gall", tag="gall")
            nc.sync.dma_start(gall[:], grid_d[s].rearrange("(c p) k -> p c k", p=128))
            gT = sb.tile([3, N], F32, name="gT", tag="gT", bufs=1)
            for ch in range(NC8):
                pt = psA((3, 128))
                nc.tensor.transpose(pt[:], gall[:, ch, :], ident[:])
                nc.vector.tensor_copy(r(gT[:, ch * 128:(ch + 1) * 128]), pt[:])
            for c in range(DC):
                for h in range(2):
                    hs = slice(h * HALF, (h + 1) * HALF)
                    acc = psA()
                    nc.tensor.matmul(acc[:], r(embw[:, c * 128:(c + 1) * 128]),
                                     r(gT[:, hs]), start=True, stop=True)
                    nc.scalar.copy(r(xT[:, c, hs]), acc[:])

        # ---- layers ----
        for l in range(L):
            wq = wp.tile([128, DC, 3 * D], F32, name="wq", tag="wq")
            nc.sync.dma_start(r(wq[:]), r(wfull[OFF_QKV + l * SZ_QKV:OFF_QKV + (l + 1) * SZ_QKV]
                                          .rearrange("(c p f) -> p c f", c=DC, p=128)))
            wo = wp.tile([128, DC, D], F32, name="wo", tag="wo")
            nc.sync.dma_start(r(wo[:]), r(wfull[OFF_WO + l * SZ_WO:OFF_WO + (l + 1) * SZ_WO]
                                          .rearrange("(c p f) -> p c f", c=DC, p=128)))
            w1 = wp.tile([128, DC, 4 * D], F32, name="w1", tag="w1")
            nc.sync.dma_start(r(w1[:]), r(wfull[OFF_W1 + l * SZ_W1:OFF_W1 + (l + 1) * SZ_W1]
                                          .rearrange("(c p f) -> p c f", c=DC, p=128)))
            w2 = wp.tile([128, 12, D], F32, name="w2", tag="w2")
            nc.sync.dma_start(r(w2[:]), r(wfull[OFF_W2 + l * SZ_W2:OFF_W2 + (l + 1) * SZ_W2]
                                          .rearrange("(c p f) -> p c f", c=12, p=128)))
            vbrow = wp.tile([1, D], F32, name="vbrow", tag="vbrow")
            nc.sync.dma_start(r(vbrow[:]), r(wfull[OFF_VB + l * SZ_VB:OFF_VB + (l + 1) * SZ_VB]
                                             .rearrange("(a b) -> a b", a=1)))
            vbB = wp.tile([128, D], F32, name="vbB", tag="vbB")
            pvb = psA((128, D))
            nc.tensor.matmul(pvb[:], r(ones1[:]), r(vbrow[:]), start=True, stop=True)
            nc.scalar.copy(vbB[:], pvb[:])
            biasL = biases[:, l, :]

            for s, xT in enumerate(xT_list):
                hT = big.tile([128, DC, N], F32, name="hT", tag="hT")
                layernorm(xT, hT)
                qkT = big.tile([128, 6, N], F32, name="qkT", tag="qkT")
                for m in range(6):
                    for h in range(2):
                        hs = slice(h * HALF, (h + 1) * HALF)
                        acc = psA()
                        for c in range(DC):
                            nc.tensor.matmul(acc[:], r(wq[:, c, m * 128:(m + 1) * 128]),
                                             r(hT[:, c, hs]), start=(c == 0), stop=(c == DC - 1))
                        nc.vector.tensor_scalar(r(qkT[:, m, hs]), acc[:], biasL[:, m:m + 1],
                                                None, op0=OP.add)
                vtok = big.tile([128, NC8, H, 65], F32, name="vtok", tag="vtok")
                nc.vector.memset(vtok[:, :, :, 64:65], 1.0)
                for t in range(NC8):
                    acc = psA((128, D))
                    for c in range(DC):
                        nc.tensor.matmul(acc[:], r(hT[:, c, t * 128:(t + 1) * 128]),
                                         r(wq[:, c, 2 * D:3 * D]), start=(c == 0), stop=(c == DC - 1))
                    nc.vector.scalar_tensor_tensor(
                        r(vtok[:, t, :, 0:64]), acc[:].rearrange("p (h d) -> p h d", h=H),
                        0.0, vbB[:].rearrange("p (h d) -> p h d", h=H),
                        op0=OP.add, op1=OP.add)
                OallT = big.tile([128, DC, N], F32, name="OallT", tag="OallT")
                for h in range(H):
                    qh = qkT[(h % 2) * 64:(h % 2) * 64 + 64, h // 2, :]
                    kh = qkT[(h % 2) * 64:(h % 2) * 64 + 64, 3 + h // 2, :]
                    pot = ps.tile([65, N], F32, name="pot", tag="pot", bufs=1)
                    for cj in range(NC8):
                        for x in range(2):
                            xs = slice(x * HALF, (x + 1) * HALF)
                            pst = psA()
                            nc.tensor.matmul(pst[:], r(kh[:, cj * 128:(cj + 1) * 128]),
                                             r(qh[:, xs]), start=True, stop=True)
                            est = sb.tile([128, HALF], F32, name="est", tag="est")
                            nc.scalar.activation(r(est[:]), pst[:], AF.Exp)
                            nc.tensor.matmul(pot[:, xs], r(vtok[:, cj, h, :]), r(est[:]),
                                             start=(cj == 0), stop=(cj == NC8 - 1))
                    rden = sb.tile([1, N], F32, name="rden", tag="rden", bufs=1)
                    nc.vector.reciprocal(r(rden[:]), pot[64:65, :])
                    rdB = sb.tile([64, N], F32, name="rdB", tag="rdB", bufs=1)
                    for x in range(2):
                        xs = slice(x * HALF, (x + 1) * HALF)
                        prb = psA((64, HALF))
                        nc.tensor.matmul(prb[:], r(ones1[:, 0:64]), r(rden[:, xs]),
                                         start=True, stop=True)
                        nc.scalar.copy(rdB[:, xs], prb[:])
                    dst = OallT[(h % 2) * 64:(h % 2) * 64 + 64, h // 2, :]
                    nc.vector.scalar_tensor_tensor(r(dst), pot[0:64, :], 0.0, rdB[:],
                                                   op0=OP.bypass, op1=OP.mult)
                for m in range(DC):
                    for x in range(2):
                        xs = slice(x * HALF, (x + 1) * HALF)
                        acc = psA()
                        for c in range(DC):
                            nc.tensor.matmul(acc[:], r(wo[:, c, m * 128:(m + 1) * 128]),
                                             r(OallT[:, c, xs]), start=(c == 0), stop=(c == DC - 1))
                        nc.vector.scalar_tensor_tensor(r(xT[:, m, xs]), acc[:],
                                                       biasL[:, 9 + m:10 + m], xT[:, m, xs],
                                                       op0=OP.add, op1=OP.add)
                hT2 = big.tile([128, DC, N], F32, name="hT2", tag="hT")
                layernorm(xT, hT2)
                for x in range(2):
                    xs = slice(x * HALF, (x + 1) * HALF)
                    pm = [ps.tile([128, HALF], F32, name=f"pm{m}", tag=f"pm{m}", bufs=1)
                          for m in range(DC)]
                    for k in range(12):
                        ph = ps.tile([128, HALF], F32, name="ph", tag="ph", bufs=1)
                        for c in range(DC):
                            nc.tensor.matmul(ph[:], r(w1[:, c, k * 128:(k + 1) * 128]),
                                             r(hT2[:, c, xs]), start=(c == 0), stop=(c == DC - 1))
                        g = sb.tile([128, HALF], F32, name="g", tag="g")
                        nc.scalar.activation(r(g[:]), ph[:], AF.Gelu_apprx_tanh,
                                             bias=biasL[:, 12 + k:13 + k], scale=1.0)
                        for m in range(DC):
                            nc.tensor.matmul(pm[m][:], r(w2[:, k, m * 128:(m + 1) * 128]),
                                             r(g[:]), start=(k == 0), stop=(k == 11))
                    for m in range(DC):
                        nc.vector.scalar_tensor_tensor(r(xT[:, m, xs]), pm[m][:],
                                                       biasL[:, 24 + m:25 + m], xT[:, m, xs],
                                                       op0=OP.add, op1=OP.add)

        # ---- proj ----
        for s in range(2):
            for h in range(2):
                hs = slice(h * HALF, (h + 1) * HALF)
                pp = psA((3, HALF))
                for c in range(DC):
                    nc.tensor.matmul(pp[:], r(projw[:, c, :]), r(xT_list[s][:, c, hs]),
                                     start=(c == 0), stop=(c == DC - 1))
                ot = sb.tile([3, HALF], F32, name="proj_ot", tag="proj_ot")
                nc.scalar.copy(ot[:], pp[:])
                nc.sync.dma_start(pred_d[s, :, hs], ot[:])
    return nc


def _run_transformer_on_device(grid, shards):
    from concourse.bass_utils import run_bass_kernel_spmd
    _install_birfix()
    if "nc" not in _NC_CACHE:
        _NC_CACHE["nc"] = _build_transformer_nc()
    nc = _NC_CACHE["nc"]
    in_maps = [dict(grid2=np.ascontiguousarray(grid[2 * c:2 * c + 2], np.float32),
                    wshard=shards[c]) for c in range(8)]
    res = run_bass_kernel_spmd(nc, in_maps, list(range(8)))
    pred = np.concatenate(
        [res.results[c]["predT"].transpose(0, 2, 1) for c in range(8)], 0)
    return np.ascontiguousarray(pred, np.float32)  # [16, 1024, 3]


# ---------------------------------------------------------------------------
# Host pieces: FPS, chamfer, homogeneity (single CPU core)
# ---------------------------------------------------------------------------

def _fps_all(pts):
    bidx = np.arange(B)
    dists = np.full((B, P), 1e10, np.float32)
    last = np.zeros(B, np.int64)
    idxs = np.empty((B, N), np.int64)
    for t in range(N):
        idxs[:, t] = last
        c = pts[bidx, last]
        diff = pts - c[:, None, :]
        d = np.sum(diff * diff, axis=-1, dtype=np.float32)
        dists = np.minimum(dists, d)
        last = np.argmax(dists, axis=1)
    return pts[bidx[:, None], idxs]


def _losses(pred, centers):
    """Chamfer L1 + homogeneity KL, vectorized per sample."""
    recs = np.empty(B, np.float32)
    kls = np.empty(B, np.float32)
    logq = np.float32(np.log(1.0 / N))
    for b in range(B):
        pb = pred[b].astype(np.float32)
        cb = centers[b].astype(np.float32)
        ppb = np.einsum("ij,ij->i", pb, pb)
        ccb = np.einsum("ij,ij->i", cb, cb)
        g = pb @ cb.T
        d2 = ppb[:, None] - 2.0 * g + ccb[None, :]
        np.maximum(d2, 0.0, out=d2)
        d = np.sqrt(d2, dtype=np.float32)
        recs[b] = np.float32(0.5) * (d.min(axis=1).mean(dtype=np.float32)
                                     + d.min(axis=0).mean(dtype=np.float32))
        g2 = pb @ pb.T
        dd2 = ppb[:, None] - 2.0 * g2 + ppb[None, :]
        np.maximum(dd2, 0.0, out=dd2)
        np.fill_diagonal(dd2, 0.0)
        dd = np.sqrt(dd2, dtype=np.float32)
        part = np.partition(dd, K_NEI, axis=-1)[:, :K_NEI + 1]
        part.sort(axis=-1)
        mean_d = part[:, 1:].mean(axis=-1, dtype=np.float32)
        m = mean_d.max()
        lse = m + np.float32(np.log(np.sum(np.exp(mean_d - m), dtype=np.float32)))
        logp = mean_d - lse
        kls[b] = np.sum(np.float32(1.0 / N) * (logq - logp), dtype=np.float32)
    return recs, kls


# ---------------------------------------------------------------------------
# Host fallback transformer (only used if the device path fails)
# ---------------------------------------------------------------------------

def _ln_np(x, w, b):
    m = np.mean(x, -1, keepdims=True, dtype=np.float32)
    v = np.mean((x - m) ** 2, -1, keepdims=True, dtype=np.float32)
    return ((x - m) / np.sqrt(v + np.float32(1e-5))) * w + b


def _gelu_tanh(x):
    c = np.float32(np.sqrt(2.0 / np.pi))
    return np.float32(0.5) * x * (np.float32(1.0)
                                  + np.tanh(c * (x + np.float32(0.044715) * x * x * x)))


def _transformer_np(x, p):
    (l1w, l1b, qw, qb, aw, ab, l2w, l2b, m1w, m1b, m2w, m2b) = p
    nb = x.shape[0]
    for l in range(L):
        h = _ln_np(x, l1w[l], l1b[l])
        qkv = (h.reshape(-1, D) @ qw[l]).reshape(nb, N, 3 * D) + qb[l]
        q, k, v = np.split(qkv, 3, axis=-1)
        rs = lambda t: t.reshape(nb, N, H, HD).transpose(0, 2, 1, 3)
        q, k, v = rs(q), rs(k), rs(v)
        s = np.matmul(q, k.transpose(0, 1, 3, 2)) / np.float32(np.sqrt(HD))
        s = s - s.max(axis=-1, keepdims=True)
        e = np.exp(s)
        att = e / e.sum(axis=-1, keepdims=True, dtype=np.float32)
        o = np.matmul(att, v).transpose(0, 2, 1, 3).reshape(nb, N, D)
        x = x + ((o.reshape(-1, D) @ aw[l]).reshape(nb, N, D) + ab[l])
        h = _ln_np(x, l2w[l], l2b[l])
        g = _gelu_tanh((h.reshape(-1, D) @ m1w[l]).reshape(nb, N, 4 * D) + m1b[l])
        x = x + ((g.reshape(-1, 4 * D) @ m2w[l]).reshape(nb, N, D) + m2b[l])
    return x.astype(np.float32)


# ---------------------------------------------------------------------------
# Entry point
# ---------------------------------------------------------------------------

def kernel(pts, grid, embed_w, proj_w, ln1_w, ln1_b, qkv_w, qkv_b,
           attn_w, attn_b, ln2_w, ln2_b, mlp_w1, mlp_b1, mlp_w2, mlp_b2):
    pts = np.asarray(pts, np.float32)
    grid = np.asarray(grid, np.float32)
    inputs = dict(embed_w=embed_w, proj_w=proj_w, ln1_w=ln1_w, ln1_b=ln1_b,
                  qkv_w=qkv_w, qkv_b=qkv_b, attn_w=attn_w, attn_b=attn_b,
                  ln2_w=ln2_w, ln2_b=ln2_b, mlp_w1=mlp_w1, mlp_b1=mlp_b1,
                  mlp_w2=mlp_w2, mlp_b2=mlp_b2)

    # FPS on host, overlapped with the device transformer call.
    fps_out = {}

    def fps_job():
        fps_out["centers"] = _fps_all(pts)

    th = threading.Thread(target=fps_job)
    th.start()
    pred = None
    try:
        shards = _fold_weights(inputs)
        try:
            pred = _run_transformer_on_device(grid, shards)
        except Exception as e:  # transient device wedge -> one retry
            print(f"kernel: device call failed ({type(e).__name__}: {e}); "
                  "retrying once", file=sys.stderr)
            pred = _run_transformer_on_device(grid, shards)
    except Exception as e:  # device unavailable -> equivalent host compute
        print(f"kernel: device path failed ({type(e).__name__}: {e}); "
              "using host fallback", file=sys.stderr)
        x = (grid @ np.asarray(embed_w, np.float32)).astype(np.float32)
        params = tuple(np.asarray(inputs[k], np.float32)
                       for k in ("ln1_w", "ln1_b", "qkv_w", "qkv_b", "attn_w",
                                 "attn_b", "ln2_w", "ln2_b", "mlp_w1", "mlp_b1",
                                 "mlp_w2", "mlp_b2"))
        x = _transformer_np(x, params)
        pred = (x @ np.asarray(proj_w, np.float32)).astype(np.float32)
    th.join()
    centers = fps_out["centers"]

    recs, kls = _losses(pred, centers)
    rec = np.float32(recs.mean(dtype=np.float32))
    kl = np.float32(kls.mean(dtype=np.float32))
    return (np.asarray(rec, np.float32), np.asarray(kl, np.float32))


# revision 5
# speedup vs baseline: 1.9058x; 1.9058x over previous
"""GridSmoother kernel for 8 trn2 NeuronCores.

Sharding: data-parallel over B (16 samples -> 2 per core). The 12-layer
transformer (the dominant compute) runs on the 8 NeuronCores as a Bass/Tile
program with fp32r matmuls; weights are uploaded sharded 8-ways and
AllGathered on-device to cut host->device transfer 8x. FPS runs on the host
overlapped with the device call; chamfer/homogeneity are vectorized numpy.

Environment notes (this container):
- walrus only accepts 1 sync-wait per instruction (2 on EventSemaphore); a
  BIR post-pass splits multi-wait instructions into EventSemaphore prefixes.
- Compiled NEFFs are content-cached under /tmp/bass_neff_cache.
"""

import json
import os
import sys
import threading

import numpy as np
from contextlib import ExitStack

sys.path.insert(0, "/opt/trn_rl_repo")

B, P, N, D, L, H = 16, 8192, 1024, 384, 12, 6
HD = D // H
K_NEI = 5
DC = D // 128
NC8 = N // 128
HALF = 512

# ---------------------------------------------------------------------------
# BIR fix: split multi-wait instructions (this container's walrus build only
# supports one sync-wait command per instruction), plus a NEFF content cache.
# ---------------------------------------------------------------------------

_NEFF_CACHE = "/tmp/bass_neff_cache"


def _split_waits_json(bir_json):
    d = json.loads(bir_json)
    for fn in d.get("functions", []):
        for bb in fn.get("blocks", []):
            out = []
            for ins in bb.get("instructions", []):
                si = ins.get("sync_info")
                waits = (si or {}).get("on_wait") or []
                cap = 2 if ins.get("opcode") == "EventSemaphore" else 1
                if len(waits) > cap:
                    extra, keep = waits[:-cap], waits[-cap:]
                    for gi in range(0, len(extra), 2):
                        out.append({
                            "debug": ins.get("debug"),
                            "engine": ins["engine"],
                            "ins": [], "outs": [],
                            "name": f"{ins['name']}-sw{gi}",
                            "opcode": "EventSemaphore",
                            "sync_info": {"on_update": [], "on_wait": extra[gi:gi+2]},
                        })
                    si["on_wait"] = keep
                out.append(ins)
            bb["instructions"] = out
    return json.dumps(d).encode()


def _install_birfix():
    import concourse.bass_utils as bu
    import concourse.bass2jax as b2j
    import hashlib
    import shutil
    if getattr(bu, "_birfix_installed", False):
        return
    orig = bu.compile_bir_kernel

    def patched(bir_json, tmpdir, neff_name="file.neff"):
        fixed = _split_waits_json(bir_json)
        os.makedirs(_NEFF_CACHE, exist_ok=True)
        key = hashlib.sha256(fixed).hexdigest()[:32]
        cpath = os.path.join(_NEFF_CACHE, key + ".neff")
        dst = os.path.join(tmpdir, neff_name)
        if os.path.exists(cpath):
            shutil.copyfile(cpath, dst)
            return dst
        out = orig(fixed, tmpdir, neff_name=neff_name)
        try:
            shutil.copyfile(out, cpath)
        except OSError:
            pass
        return out

    bu.compile_bir_kernel = patched
    b2j.compile_bir_kernel = patched
    bu._birfix_installed = True


# ---------------------------------------------------------------------------
# Device program: AllGather weights + embed + 12 transformer layers + proj
# ---------------------------------------------------------------------------

SZ_QKV = D * 3 * D
SZ_WO = D * D
SZ_W1 = D * 4 * D
SZ_W2 = 4 * D * D
SZ_BIAS = 128 * 27
SZ_VB = D
OFF_QKV = 0
OFF_WO = OFF_QKV + L * SZ_QKV
OFF_W1 = OFF_WO + L * SZ_WO
OFF_W2 = OFF_W1 + L * SZ_W1
OFF_BIAS = OFF_W2 + L * SZ_W2
OFF_VB = OFF_BIAS + L * SZ_BIAS
OFF_EMB = OFF_VB + L * SZ_VB
OFF_PROJ = OFF_EMB + 3 * D
TOT_RAW = OFF_PROJ + D * 3
TOT = ((TOT_RAW + 1023) // 1024) * 1024
SHARD = TOT // 8

_NC_CACHE = {}


def _fold_weights(inputs):
    """Fold LN scales into matmul weights, 1/sqrt(HD) into q."""
    f64 = lambda x: np.asarray(x, np.float64)
    ln1w, ln1b = f64(inputs["ln1_w"]), f64(inputs["ln1_b"])
    ln2w, ln2b = f64(inputs["ln2_w"]), f64(inputs["ln2_b"])
    qkvw, qkvb = f64(inputs["qkv_w"]), f64(inputs["qkv_b"])
    m1w, m1b = f64(inputs["mlp_w1"]), f64(inputs["mlp_b1"])
    Wq = ln1w[:, :, None] * qkvw
    bq = np.einsum("ld,lde->le", ln1b, qkvw) + qkvb
    Wq[:, :, :D] *= 0.125
    bq[:, :D] *= 0.125
    W1 = ln2w[:, :, None] * m1w
    b1 = np.einsum("ld,lde->le", ln2b, m1w) + m1b
    Wo = f64(inputs["attn_w"]); ab = f64(inputs["attn_b"])
    W2 = f64(inputs["mlp_w2"]); b2 = f64(inputs["mlp_b2"])
    bias_all = np.zeros((L, 128, 27), np.float64)
    for l in range(L):
        cols = np.concatenate([bq[l], ab[l], b1[l], b2[l]])
        bias_all[l] = cols.reshape(27, 128).T
    blob = np.zeros(TOT, np.float32)

    def put(off, a):
        a = np.ascontiguousarray(np.asarray(a, np.float32)).ravel()
        blob[off:off + a.size] = a

    put(OFF_QKV, Wq); put(OFF_WO, Wo); put(OFF_W1, W1); put(OFF_W2, W2)
    put(OFF_BIAS, bias_all); put(OFF_VB, bq[:, 2 * D:3 * D])
    put(OFF_EMB, inputs["embed_w"]); put(OFF_PROJ, inputs["proj_w"])
    return blob.reshape(8, SHARD)


def _build_transformer_nc():
    import concourse.bass as bass
    import concourse.tile as tile
    from concourse import mybir
    from concourse import masks

    F32 = mybir.dt.float32
    F32R = mybir.dt.float32r
    AF = mybir.ActivationFunctionType
    OP = mybir.AluOpType

    def r(t):
        return t.bitcast(F32R)

    nc = bass.Bass("TRN2", target_bir_lowering=False, debug=False, num_devices=8)
    grid_d = nc.dram_tensor("grid2", [2, N, 3], F32, kind="ExternalInput").ap()
    wsh_d = nc.dram_tensor("wshard", [SHARD], F32, kind="ExternalInput").ap()
    pred_d = nc.dram_tensor("predT", [2, 3, N], F32, kind="ExternalOutput").ap()
    wfull = nc.dram_tensor("wfull", [TOT], F32, kind="Internal",
                           addr_space="Shared").ap()
    wtmp = nc.dram_tensor("wtmp", [SHARD], F32, kind="Internal").ap()

    with tile.TileContext(nc) as tc, ExitStack() as ctx, \
            nc.allow_low_precision(reason="f32r rounding for fast matmul is intended"):
        nc.sync.dma_start(wtmp[:], wsh_d[:])
        nc.gpsimd.collective_compute(
            "AllGather", mybir.AluOpType.bypass,
            replica_groups=[[0, 1, 2, 3, 4, 5, 6, 7]],
            ins=[wtmp[:].opt()], outs=[wfull[:].opt()])
        cst = ctx.enter_context(tc.tile_pool(name="cst", bufs=1))
        sb = ctx.enter_context(tc.tile_pool(name="sb", bufs=2))
        ps = ctx.enter_context(tc.tile_pool(name="ps", bufs=2, space="PSUM"))
        big = ctx.enter_context(tc.tile_pool(name="big", bufs=1))
        wp = ctx.enter_context(tc.tile_pool(name="wpool", bufs=1))
        xp = ctx.enter_context(tc.tile_pool(name="xpool", bufs=1))

        ident = cst.tile([128, 128], F32)
        masks.make_identity(nc, ident[:])
        ones1 = cst.tile([1, 128], F32)
        nc.vector.memset(ones1[:], 1.0)
        ones128 = cst.tile([128, 1], F32)
        nc.vector.memset(ones128[:], 1.0)
        eps1 = cst.tile([1, 1], F32)
        nc.vector.memset(eps1[:], 1e-5)

        def psA(shape=(128, HALF)):
            return ps.tile(list(shape), F32, name="psA", tag="psA")

        embw = cst.tile([3, D], F32)
        nc.sync.dma_start(r(embw[:]), r(wfull[OFF_EMB:OFF_EMB + 3 * D]
                                        .rearrange("(a b) -> a b", a=3)))
        projw = cst.tile([128, DC, 3], F32)
        nc.sync.dma_start(r(projw[:]), r(wfull[OFF_PROJ:OFF_PROJ + D * 3]
                                         .rearrange("(c p k) -> p c k", c=DC, p=128)))

        def layernorm(xT, hT):
            sq = big.tile([128, DC, N], F32, name="ln_sq", tag="ln_sq")
            nc.vector.tensor_tensor(r(sq[:]), xT[:], xT[:], OP.mult)
            mean = sb.tile([1, N], F32, name="ln_mean", tag="ln_mean", bufs=1)
            veps = sb.tile([1, N], F32, name="ln_veps", tag="ln_veps", bufs=1)
            for h in range(2):
                hs = slice(h * HALF, (h + 1) * HALF)
                s1 = psA((1, HALF))
                s2 = psA((1, HALF))
                for c in range(DC):
                    nc.tensor.matmul(s1[:], r(ones128[:]), r(xT[:, c, hs]),
                                     start=(c == 0), stop=(c == DC - 1))
                    nc.tensor.matmul(s2[:], r(ones128[:]), r(sq[:, c, hs]),
                                     start=(c == 0), stop=(c == DC - 1))
                nc.vector.tensor_scalar(r(mean[:, hs]), s1[:], 1.0 / D, None, op0=OP.mult)
                msq = sb.tile([1, HALF], F32, name="ln_msq", tag="ln_msq", bufs=1)
                nc.vector.tensor_tensor(msq[:], mean[:, hs], mean[:, hs], OP.mult)
                nc.vector.scalar_tensor_tensor(veps[:, hs], s2[:], 1.0 / D, msq[:],
                                               op0=OP.mult, op1=OP.subtract)
            stdev = sb.tile([1, N], F32, name="ln_stdev", tag="ln_stdev", bufs=1)
            nc.scalar.activation(stdev[:], veps[:], AF.Sqrt, bias=eps1[0:1, 0:1], scale=1.0)
            inv = sb.tile([1, N], F32, name="ln_inv", tag="ln_inv", bufs=1)
            nc.vector.reciprocal(r(inv[:]), stdev[:])
            for h in range(2):
                hs = slice(h * HALF, (h + 1) * HALF)
                pmean = psA()
                nc.tensor.matmul(pmean[:], r(ones1[:]), r(mean[:, hs]), start=True, stop=True)
                for c in range(DC):
                    nc.vector.tensor_tensor(r(hT[:, c, hs]), xT[:, c, hs], pmean[:], OP.subtract)
                pinv = psA()
                nc.tensor.matmul(pinv[:], r(ones1[:]), r(inv[:, hs]), start=True, stop=True)
                for c in range(DC):
                    nc.vector.tensor_tensor(r(hT[:, c, hs]), hT[:, c, hs], pinv[:], OP.mult)

        # ---- embed both samples ----
        xT_list = []
        for s in range(2):
            xT = xp.tile([128, DC, N], F32, name=f"xT{s}", tag=f"xT{s}")
            xT_list.append(xT)
            gall = sb.tile([128, 8, 3], F32, name="gall", tag="gall")
            nc.sync.dma_start(gall[:], grid_d[s].rearrange("(c p) k -> p c k", p=128))
            gT = sb.tile([3, N], F32, name="gT", tag="gT", bufs=1)
            for ch in range(NC8):
                pt = psA((3, 128))
                nc.tensor.transpose(pt[:], gall[:, ch, :], ident[:])
                nc.vector.tensor_copy(r(gT[:, ch * 128:(ch + 1) * 128]), pt[:])
            for c in range(DC):
                for h in range(2):
                    hs = slice(h * HALF, (h + 1) * HALF)
                    acc = psA()
                    nc.tensor.matmul(acc[:], r(embw[:, c * 128:(c + 1) * 128]),
                                     r(gT[:, hs]), start=True, stop=True)
                    nc.scalar.copy(r(xT[:, c, hs]), acc[:])

        # ---- layers (hardware loop: program stays small) ----
        with tc.For_i(0, L) as lv:
            wq = wp.tile([128, DC, 3 * D], F32, name="wq", tag="wq")
            nc.sync.dma_start(r(wq[:]), r(wfull[bass.ds(OFF_QKV + lv * SZ_QKV, SZ_QKV)]
                                          .rearrange("(c p f) -> p c f", c=DC, p=128)))
            wo = wp.tile([128, DC, D], F32, name="wo", tag="wo")
            nc.sync.dma_start(r(wo[:]), r(wfull[bass.ds(OFF_WO + lv * SZ_WO, SZ_WO)]
                                          .rearrange("(c p f) -> p c f", c=DC, p=128)))
            w1 = wp.tile([128, DC, 4 * D], F32, name="w1", tag="w1")
            nc.sync.dma_start(r(w1[:]), r(wfull[bass.ds(OFF_W1 + lv * SZ_W1, SZ_W1)]
                                          .rearrange("(c p f) -> p c f", c=DC, p=128)))
            w2 = wp.tile([128, 12, D], F32, name="w2", tag="w2")
            nc.sync.dma_start(r(w2[:]), r(wfull[bass.ds(OFF_W2 + lv * SZ_W2, SZ_W2)]
                                          .rearrange("(c p f) -> p c f", c=12, p=128)))
            vbrow = wp.tile([1, D], F32, name="vbrow", tag="vbrow")
            nc.sync.dma_start(r(vbrow[:]), r(wfull[bass.ds(OFF_VB + lv * SZ_VB, SZ_VB)]
                                             .rearrange("(a b) -> a b", a=1)))
            biasT = wp.tile([128, 27], F32, name="biasT", tag="biasT")
            nc.sync.dma_start(biasT[:], wfull[bass.ds(OFF_BIAS + lv * SZ_BIAS, SZ_BIAS)]
                              .rearrange("(p c) -> p c", p=128))
            vbB = wp.tile([128, D], F32, name="vbB", tag="vbB")
            pvb = psA((128, D))
            nc.tensor.matmul(pvb[:], r(ones1[:]), r(vbrow[:]), start=True, stop=True)
            nc.scalar.copy(vbB[:], pvb[:])
            biasL = biasT

            for s, xT in enumerate(xT_list):
                hT = big.tile([128, DC, N], F32, name="hT", tag="hT")
                layernorm(xT, hT)
                qkT = big.tile([128, 6, N], F32, name="qkT", tag="qkT")
                for m in range(6):
                    for h in range(2):
                        hs = slice(h * HALF, (h + 1) * HALF)
                        acc = psA()
                        for c in range(DC):
                            nc.tensor.matmul(acc[:], r(wq[:, c, m * 128:(m + 1) * 128]),
                                             r(hT[:, c, hs]), start=(c == 0), stop=(c == DC - 1))
                        nc.vector.tensor_scalar(r(qkT[:, m, hs]), acc[:], biasL[:, m:m + 1],
                                                None, op0=OP.add)
                vtok = big.tile([128, NC8, H, 65], F32, name="vtok", tag="vtok")
                nc.vector.memset(vtok[:, :, :, 64:65], 1.0)
                for t in range(NC8):
                    acc = psA((128, D))
                    for c in range(DC):
                        nc.tensor.matmul(acc[:], r(hT[:, c, t * 128:(t + 1) * 128]),
                                         r(wq[:, c, 2 * D:3 * D]), start=(c == 0), stop=(c == DC - 1))
                    nc.vector.scalar_tensor_tensor(
                        r(vtok[:, t, :, 0:64]), acc[:].rearrange("p (h d) -> p h d", h=H),
                        0.0, vbB[:].rearrange("p (h d) -> p h d", h=H),
                        op0=OP.add, op1=OP.add)
                OallT = big.tile([128, DC, N], F32, name="OallT", tag="OallT")
                for h in range(H):
                    qh = qkT[(h % 2) * 64:(h % 2) * 64 + 64, h // 2, :]
                    kh = qkT[(h % 2) * 64:(h % 2) * 64 + 64, 3 + h // 2, :]
                    pot = ps.tile([65, N], F32, name="pot", tag="pot", bufs=1)
                    for cj in range(NC8):
                        for x in range(2):
                            xs = slice(x * HALF, (x + 1) * HALF)
                            pst = psA()
                            nc.tensor.matmul(pst[:], r(kh[:, cj * 128:(cj + 1) * 128]),
                                             r(qh[:, xs]), start=True, stop=True)
                            est = sb.tile([128, HALF], F32, name="est", tag="est")
                            nc.scalar.activation(r(est[:]), pst[:], AF.Exp)
                            nc.tensor.matmul(pot[:, xs], r(vtok[:, cj, h, :]), r(est[:]),
                                             start=(cj == 0), stop=(cj == NC8 - 1))
                    rden = sb.tile([1, N], F32, name="rden", tag="rden", bufs=1)
                    nc.vector.reciprocal(r(rden[:]), pot[64:65, :])
                    rdB = sb.tile([64, N], F32, name="rdB", tag="rdB", bufs=1)
                    for x in range(2):
                        xs = slice(x * HALF, (x + 1) * HALF)
                        prb = psA((64, HALF))
                        nc.tensor.matmul(prb[:], r(ones1[:, 0:64]), r(rden[:, xs]),
                                         start=True, stop=True)
                        nc.scalar.copy(rdB[:, xs], prb[:])
                    dst = OallT[(h % 2) * 64:(h % 2) * 64 + 64, h // 2, :]
                    nc.vector.scalar_tensor_tensor(r(dst), pot[0:64, :], 0.0, rdB[:],
                                                   op0=OP.bypass, op1=OP.mult)
                for m in range(DC):
                    for x in range(2):
                        xs = slice(x * HALF, (x + 1) * HALF)
                        acc = psA()
                        for c in range(DC):
                            nc.tensor.matmul(acc[:], r(wo[:, c, m * 128:(m + 1) * 128]),
                                             r(OallT[:, c, xs]), start=(c == 0), stop=(c == DC - 1))
                        nc.vector.scalar_tensor_tensor(r(xT[:, m, xs]), acc[:],
                                                       biasL[:, 9 + m:10 + m], xT[:, m, xs],
                                                       op0=OP.add, op1=OP.add)
                hT2 = big.tile([128, DC, N], F32, name="hT2", tag="hT")
                layernorm(xT, hT2)
                for x in range(2):
                    xs = slice(x * HALF, (x + 1) * HALF)
                    pm = [ps.tile([128, HALF], F32, name=f"pm{m}", tag=f"pm{m}", bufs=1)
                          for m in range(DC)]
                    for k in range(12):
                        ph = ps.tile([128, HALF], F32, name="ph", tag="ph", bufs=1)
                        for c in range(DC):
                            nc.tensor.matmul(ph[:], r(w1[:, c, k * 128:(k + 1) * 128]),
                                             r(hT2[:, c, xs]), start=(c == 0), stop=(c == DC - 1))
                        g = sb.tile([128, HALF], F32, name="g", tag="g")
                        nc.scalar.activation(r(g[:]), ph[:], AF.Gelu_apprx_tanh,
                                             bias=biasL[:, 12 + k:13 + k], scale=1.0)
                        for m in range(DC):
                            nc.tensor.matmul(pm[m][:], r(w2[:, k, m * 128:(m + 1) * 128]),
                                             r(g[:]), start=(k == 0), stop=(k == 11))
                    for m in range(DC):
                        nc.vector.scalar_tensor_tensor(r(xT[:, m, xs]), pm[m][:],
                                                       biasL[:, 24 + m:25 + m], xT[:, m, xs],
                                                       op0=OP.add, op1=OP.add)

        # ---- proj ----
        for s in range(2):
            for h in range(2):
                hs = slice(h * HALF, (h + 1) * HALF)
                pp = psA((3, HALF))
                for c in range(DC):
                    nc.tensor.matmul(pp[:], r(projw[:, c, :]), r(xT_list[s][:, c, hs]),
                                     start=(c == 0), stop=(c == DC - 1))
                ot = sb.tile([3, HALF], F32, name="proj_ot", tag="proj_ot")
                nc.scalar.copy(ot[:], pp[:])
                nc.sync.dma_start(pred_d[s, :, hs], ot[:])
    return nc


def _run_transformer_on_device(grid, shards):
    from concourse.bass_utils import run_bass_kernel_spmd
    _install_birfix()
    if "nc" not in _NC_CACHE:
        _NC_CACHE["nc"] = _build_transformer_nc()
    nc = _NC_CACHE["nc"]
    in_maps = [dict(grid2=np.ascontiguousarray(grid[2 * c:2 * c + 2], np.float32),
                    wshard=shards[c]) for c in range(8)]
    res = run_bass_kernel_spmd(nc, in_maps, list(range(8)))
    pred = np.concatenate(
        [res.results[c]["predT"].transpose(0, 2, 1) for c in range(8)], 0)
    return np.ascontiguousarray(pred, np.float32)  # [16, 1024, 3]


# ---------------------------------------------------------------------------
# Host pieces: FPS, chamfer, homogeneity (single CPU core)
# ---------------------------------------------------------------------------

def _fps_all(pts):
    bidx = np.arange(B)
    dists = np.full((B, P), 1e10, np.float32)
    last = np.zeros(B, np.int64)
    idxs = np.empty((B, N), np.int64)
    for t in range(N):
        idxs[:, t] = last
        c = pts[bidx, last]
        diff = pts - c[:, None, :]
        d = np.sum(diff * diff, axis=-1, dtype=np.float32)
        dists = np.minimum(dists, d)
        last = np.argmax(dists, axis=1)
    return pts[bidx[:, None], idxs]


def _losses(pred, centers):
    """Chamfer L1 + homogeneity KL, vectorized per sample."""
    recs = np.empty(B, np.float32)
    kls = np.empty(B, np.float32)
    logq = np.float32(np.log(1.0 / N))
    for b in range(B):
        pb = pred[b].astype(np.float32)
        cb = centers[b].astype(np.float32)
        ppb = np.einsum("ij,ij->i", pb, pb)
        ccb = np.einsum("ij,ij->i", cb, cb)
        g = pb @ cb.T
        d2 = ppb[:, None] - 2.0 * g + ccb[None, :]
        np.maximum(d2, 0.0, out=d2)
        d = np.sqrt(d2, dtype=np.float32)
        recs[b] = np.float32(0.5) * (d.min(axis=1).mean(dtype=np.float32)
                                     + d.min(axis=0).mean(dtype=np.float32))
        g2 = pb @ pb.T
        dd2 = ppb[:, None] - 2.0 * g2 + ppb[None, :]
        np.maximum(dd2, 0.0, out=dd2)
        np.fill_diagonal(dd2, 0.0)
        dd = np.sqrt(dd2, dtype=np.float32)
        part = np.partition(dd, K_NEI, axis=-1)[:, :K_NEI + 1]
        part.sort(axis=-1)
        mean_d = part[:, 1:].mean(axis=-1, dtype=np.float32)
        m = mean_d.max()
        lse = m + np.float32(np.log(np.sum(np.exp(mean_d - m), dtype=np.float32)))
        logp = mean_d - lse
        kls[b] = np.sum(np.float32(1.0 / N) * (logq - logp), dtype=np.float32)
    return recs, kls


# ---------------------------------------------------------------------------
# Host fallback transformer (only used if the device path fails)
# ---------------------------------------------------------------------------

def _ln_np(x, w, b):
    m = np.mean(x, -1, keepdims=True, dtype=np.float32)
    v = np.mean((x - m) ** 2, -1, keepdims=True, dtype=np.float32)
    return ((x - m) / np.sqrt(v + np.float32(1e-5))) * w + b


def _gelu_tanh(x):
    c = np.float32(np.sqrt(2.0 / np.pi))
    return np.float32(0.5) * x * (np.float32(1.0)
                                  + np.tanh(c * (x + np.float32(0.044715) * x * x * x)))


def _transformer_np(x, p):
    (l1w, l1b, qw, qb, aw, ab, l2w, l2b, m1w, m1b, m2w, m2b) = p
    nb = x.shape[0]
    for l in range(L):
        h = _ln_np(x, l1w[l], l1b[l])
        qkv = (h.reshape(-1, D) @ qw[l]).reshape(nb, N, 3 * D) + qb[l]
        q, k, v = np.split(qkv, 3, axis=-1)
        rs = lambda t: t.reshape(nb, N, H, HD).transpose(0, 2, 1, 3)
        q, k, v = rs(q), rs(k), rs(v)
        s = np.matmul(q, k.transpose(0, 1, 3, 2)) / np.float32(np.sqrt(HD))
        s = s - s.max(axis=-1, keepdims=True)
        e = np.exp(s)
        att = e / e.sum(axis=-1, keepdims=True, dtype=np.float32)
        o = np.matmul(att, v).transpose(0, 2, 1, 3).reshape(nb, N, D)
        x = x + ((o.reshape(-1, D) @ aw[l]).reshape(nb, N, D) + ab[l])
        h = _ln_np(x, l2w[l], l2b[l])
        g = _gelu_tanh((h.reshape(-1, D) @ m1w[l]).reshape(nb, N, 4 * D) + m1b[l])
        x = x + ((g.reshape(-1, 4 * D) @ m2w[l]).reshape(nb, N, D) + m2b[l])
    return x.astype(np.float32)


# ---------------------------------------------------------------------------
# Entry point
# ---------------------------------------------------------------------------

def kernel(pts, grid, embed_w, proj_w, ln1_w, ln1_b, qkv_w, qkv_b,
           attn_w, attn_b, ln2_w, ln2_b, mlp_w1, mlp_b1, mlp_w2, mlp_b2):
    pts = np.asarray(pts, np.float32)
    grid = np.asarray(grid, np.float32)
    inputs = dict(embed_w=embed_w, proj_w=proj_w, ln1_w=ln1_w, ln1_b=ln1_b,
                  qkv_w=qkv_w, qkv_b=qkv_b, attn_w=attn_w, attn_b=attn_b,
                  ln2_w=ln2_w, ln2_b=ln2_b, mlp_w1=mlp_w1, mlp_b1=mlp_b1,
                  mlp_w2=mlp_w2, mlp_b2=mlp_b2)

    # FPS on host, overlapped with the device transformer call.
    fps_out = {}

    def fps_job():
        fps_out["centers"] = _fps_all(pts)

    th = threading.Thread(target=fps_job)
    th.start()
    pred = None
    try:
        shards = _fold_weights(inputs)
        try:
            pred = _run_transformer_on_device(grid, shards)
        except Exception as e:  # transient device wedge -> one retry
            print(f"kernel: device call failed ({type(e).__name__}: {e}); "
                  "retrying once", file=sys.stderr)
            pred = _run_transformer_on_device(grid, shards)
    except Exception as e:  # device unavailable -> equivalent host compute
        print(f"kernel: device path failed ({type(e).__name__}: {e}); "
              "using host fallback", file=sys.stderr)
        x = (grid @ np.asarray(embed_w, np.float32)).astype(np.float32)
        params = tuple(np.asarray(inputs[k], np.float32)
                       for k in ("ln1_w", "ln1_b", "qkv_w", "qkv_b", "attn_w",
                                 "attn_b", "ln2_w", "ln2_b", "mlp_w1", "mlp_b1",
                                 "mlp_w2", "mlp_b2"))
        x = _transformer_np(x, params)
        pred = (x @ np.asarray(proj_w, np.float32)).astype(np.float32)
    th.join()
    centers = fps_out["centers"]

    recs, kls = _losses(pred, centers)
    rec = np.float32(recs.mean(dtype=np.float32))
    kl = np.float32(kls.mean(dtype=np.float32))
    return (np.asarray(rec, np.float32), np.asarray(kl, np.float32))
